# revision 13
# baseline (speedup 1.0000x reference)
"""Trainium2 Bass kernel for nn_AttentionLayer (additive attention pooling).

reference math:
    re = entities @ w1_w + w1_b                  # [B, H]
    rc = contexts @ w2_w + w2_b                  # [B, S, H]
    scores = tanh(re[:,None,:] + rc) @ v_w + v_b # [B, S, 1]
    weights = softmax(scores, axis=1)
    out = weights * contexts                     # [B, S, D]

Sharding: data-parallel over B across 8 cores (4 batches/core), weights
replicated.

Numerics: all matmuls (rc, re, score matvec) run in fp8 e4m3 with DoubleRow
perf mode (2 MACs/cell/cycle, f32 accumulation).  Small-magnitude operands
(w1, w2, v) are pre-scaled by 2^8 on the host so their values sit in the fp8
normal range; the consuming activation rescales by 2^-8.  tanh/softmax math
is f32; the softmax is shift-invariant so v_b is dropped.  Output is
produced bf16 and upcast on the host.  Measured end-to-end rel err ~1.4e-2.

Layout: the host pre-transposes contexts to [d, tokens] with the DoubleRow
pair interleave (d = ko2*256 + two*128 + Ki) and pre-casts all operands, so
the device does no transposes or weight casts at all.

Schedule: per chunk of 512 tokens, 8 ho-groups x 4 DoubleRow MMs feed PSUM;
tanh (ACT) drains each group to SBUF fp8 pair tiles.  The score matvecs for
chunk T are emitted as one block during chunk T+1, and the softmax /
weight-transpose / output-scale stages of batch b are emitted during batch
b+1, so the in-order PE queue never waits on ACT/DVE producers.  The weight
transpose bounces scores through DRAM (free transpose) instead of using PE.
DMA rings are split by traffic class: inputs on the Sync-engine ring,
outputs on GpSimd, score bounces on Vector, so small latency-critical
transfers never queue behind megabyte bursts.  A short warmup MM spin at
the head brings the PE out of its HAM half-clock state while input DMAs
are still in flight.
"""

import sys

for _p in ("/opt/trn_rl_repo", "/root/.axon_site/_ro/trn_rl_repo"):
    if _p not in sys.path:
        sys.path.insert(0, _p)

import numpy as np
import ml_dtypes

B, S, D, H = 32, 2048, 1024, 1024
N_CORES = 8
B_LOC = B // N_CORES          # batches per core
P = 128
TCHUNK = 512                  # tokens per main-loop chunk
NC = S // TCHUNK              # chunks per batch
NT = S // P                   # 128-token tiles per batch
KO2 = D // 256                # DoubleRow k-tiles (256 contraction each)
HO = H // P
HO2 = HO // 2                 # score matvec DoubleRow pairs
TOK = B_LOC * S               # tokens per core

F8 = ml_dtypes.float8_e4m3
BF16 = ml_dtypes.bfloat16
FP8_SCALE = 256.0             # host multiplies w1/w2/v by this before fp8 cast


def build_attention(tc, out_ap, ins, b_loc=B_LOC):
    """Emit the per-core kernel into TileContext `tc`.

    out_ap: DRAM AP [b_loc*S, D] bf16
    ins: DRAM APs:
      xt8   [b_loc*NC*P, KO2*2*TCHUNK] f8e4   transposed ctx, chunk-blocked
      xbf   [b_loc*S, D] bf16                 contexts (final multiply)
      w2dr  [P, KO2, 2, H] f8e4               w2 * 256, DoubleRow interleave
      w1dr  [P, KO2, 2, H] f8e4               w1 * 256, DoubleRow interleave
      ent8  [P, KO2, 2, 16] f8e4              entities^T in [..., :b_loc], 16-padded
      v8    [P, HO2, 2, 16] f8e4              v * 256 in [..., 0], 16-padded
      biasb [P, HO] f32                       w1_b + w2_b, h on partitions
    """
    from contextlib import ExitStack

    import concourse.mybir as mybir
    from concourse.masks import make_identity

    nc = tc.nc
    f32 = mybir.dt.float32
    bf16 = mybir.dt.bfloat16
    f8e4 = mybir.dt.float8e4
    AF = mybir.ActivationFunctionType
    DR = mybir.MatmulPerfMode.DoubleRow
    EP = 32                       # padded partition count for re transpose
    INV = 1.0 / FP8_SCALE

    xt8c = ins["xt8"].rearrange("(c p) f -> c p f", p=P)    # [b_loc*NC, P, 4KB]
    xbf3 = ins["xbf"].rearrange("(n p) dd -> n p dd", p=P)  # [b_loc*NT, P, D]
    out3 = out_ap.rearrange("(n p) dd -> n p dd", p=P)

    with ExitStack() as ctx:
        consts = ctx.enter_context(tc.tile_pool(name="consts", bufs=1))
        wpool = ctx.enter_context(tc.tile_pool(name="wpool", bufs=1))

        ps_rc = ctx.enter_context(tc.tile_pool(name="ps_rc", bufs=5, space="PSUM"))
        ps_sc = ctx.enter_context(tc.tile_pool(name="ps_sc", bufs=2, space="PSUM"))
        ps_misc = ctx.enter_context(tc.tile_pool(name="ps_misc", bufs=1, space="PSUM"))
        scr_pool = ctx.enter_context(tc.tile_pool(name="scr", bufs=2, space="DRAM"))

        # ---------------- constants / weights ----------------
        id32 = consts.tile([EP, EP], f32, tag="id32")
        make_identity(nc, id32)
        ones_col = consts.tile([P, 1], f32, tag="ones_col")
        nc.vector.memset(ones_col, 1.0)
        ones_row = consts.tile([1, P], f32, tag="ones_row")
        nc.vector.memset(ones_row, 1.0)
        warm_sb = consts.tile([P, 64], bf16, tag="warm")
        nc.vector.memset(warm_sb, 0.125)

        # small inputs first so the entity path can start early; the big
        # context loads queue up behind them on the Sync ring
        biasb_sb = consts.tile([P, HO], f32, tag="biasb")
        v8_sb = consts.tile([P, HO2, 2, 16], f8e4, tag="v8")
        ent8_sb = consts.tile([P, KO2, 2, 16], f8e4, tag="ent8")
        nc.sync.dma_start(out=biasb_sb, in_=ins["biasb"])
        nc.sync.dma_start(out=v8_sb, in_=ins["v8"])
        nc.sync.dma_start(out=ent8_sb, in_=ins["ent8"])

        w1dr_sb = consts.tile([P, KO2, 2, H], f8e4, tag="w1dr")
        nc.sync.dma_start(out=w1dr_sb, in_=ins["w1dr"])
        w2dr_sb = wpool.tile([P, KO2, 2, H], f8e4, tag="w2dr")
        nc.sync.dma_start(out=w2dr_sb, in_=ins["w2dr"])

        # ---------------- main-loop pools ----------------
        xt8_pool = ctx.enter_context(tc.tile_pool(name="xt8", bufs=2))
        xbf_pool = ctx.enter_context(tc.tile_pool(name="xbf", bufs=2))
        th_pool = ctx.enter_context(tc.tile_pool(name="th", bufs=3))
        sw_pool = ctx.enter_context(tc.tile_pool(name="sw", bufs=3))
        out_pool = ctx.enter_context(tc.tile_pool(name="outp", bufs=4))

        # ---------------- PE warmup spin ----------------
        # ~64 tiny matmuls keep the PE busy while input DMAs land, pushing
        # the HAM clock gate to 8/8 before the real work starts
        for w in range(64):
            warm_ps = ps_misc.tile([64, 64], f32, tag="misc")
            nc.tensor.matmul(
                warm_ps, lhsT=warm_sb[:, :64], rhs=warm_sb, start=True, stop=True
            )

        # ---------------- entities path (fp8 DoubleRow) ----------------
        # re[b, h] = entities @ w1; reb_sb[:, ho, b] = re^T + (w1_b + w2_b)
        re_sb = consts.tile([EP, H], f32, tag="re_sb")
        nc.vector.memset(re_sb, 0.0)
        hc = 512
        for n0 in range(0, H, hc):
            re_ps = ps_misc.tile([b_loc, hc], f32, tag="misc")
            for ko2 in range(KO2):
                nc.tensor.matmul(
                    re_ps,
                    lhsT=ent8_sb[:, ko2, :, 0:b_loc],
                    rhs=w1dr_sb[:, ko2, :, n0 : n0 + hc],
                    start=(ko2 == 0),
                    stop=(ko2 == KO2 - 1),
                    perf_mode=DR,
                )
            nc.scalar.activation(
                out=re_sb[:b_loc, n0 : n0 + hc], in_=re_ps, func=AF.Copy, scale=INV
            )

        reb_sb = consts.tile([P, HO, b_loc], f32, tag="reb")
        for ho in range(HO):
            rtr = ps_misc.tile([P, EP], f32, tag="misc")
            nc.tensor.transpose(rtr, re_sb[:, ho * P : (ho + 1) * P], id32)
            nc.vector.tensor_scalar(
                out=reb_sb[:, ho, :],
                in0=rtr[:, :b_loc],
                scalar1=biasb_sb[:, ho : ho + 1],
                scalar2=None,
                op0=mybir.AluOpType.add,
            )

        # ---------------- pipelined main loop ----------------
        NCH = b_loc * NC          # total chunks
        xt8_sb = [None] * b_loc   # [P, KO2, 2, S] per batch
        xbf_sb = [None] * b_loc   # [P, NT, D] per batch
        sw = [None] * b_loc       # [1, S] scores (x256) per batch
        scr = [None] * b_loc      # DRAM bounce tiles for score transpose
        wT = [None] * b_loc       # [P, NT] transposed weights per batch
        th_tiles = {}             # chunk -> [P, 2, HO2, TCHUNK] fp8 tile

        def dma_in_batch(b):
            xt8_sb[b] = xt8_pool.tile([P, KO2, 2, S], f8e4, tag="xt8", name=f"xt8sb{b}")
            for T in range(NC):
                c = b * NC + T
                nc.sync.dma_start(
                    out=xt8_sb[b][:, :, :, T * TCHUNK : (T + 1) * TCHUNK],
                    in_=xt8c[c : c + 1].rearrange(
                        "n p (k two t) -> p (n k) two t", k=KO2, two=2
                    ),
                )
            xbf_sb[b] = xbf_pool.tile([P, NT, D], bf16, tag="xbf", name=f"xbfsb{b}")
            for t4 in range(0, NT, 4):
                nc.sync.dma_start(
                    out=xbf_sb[b][:, t4 : t4 + 4, :],
                    in_=xbf3[b * NT + t4 : b * NT + t4 + 4].rearrange(
                        "n p dd -> p n dd"
                    ),
                )

        def emit_rc_group(b, T, ho):
            rc_ps = ps_rc.tile([P, TCHUNK], f32, tag="rc")
            for ko2 in range(KO2):
                nc.tensor.matmul(
                    rc_ps,
                    lhsT=w2dr_sb[:, ko2, :, ho * P : (ho + 1) * P],
                    rhs=xt8_sb[b][:, ko2, :, T * TCHUNK : (T + 1) * TCHUNK],
                    start=(ko2 == 0),
                    stop=(ko2 == KO2 - 1),
                    perf_mode=DR,
                )
            j, half = divmod(ho, 2)
            if ho == 0:
                th_tiles[b * NC + T] = th_pool.tile(
                    [P, 2, HO2, TCHUNK], f8e4, tag="th", name=f"th{b}_{T}"
                )
            nc.scalar.activation(
                out=th_tiles[b * NC + T][:, half, j, :],
                in_=rc_ps,
                func=AF.Tanh,
                bias=reb_sb[:, ho, b : b + 1],
                scale=INV,
            )

        def emit_matvec_block(c):
            # fp8 DoubleRow score matvecs for chunk c (deferred one chunk);
            # produces scores x256 (v is host-scaled), undone at the exp
            sc_ps = ps_sc.tile([1, TCHUNK], f32, tag="sc", name=f"scps{c}")
            for j in range(HO2):
                nc.tensor.matmul(
                    sc_ps,
                    lhsT=v8_sb[:, j, :, 0:1],
                    rhs=th_tiles[c][:, :, j, :],
                    start=(j == 0),
                    stop=(j == HO2 - 1),
                    perf_mode=DR,
                )
            b, T = divmod(c, NC)
            if T == 0:
                sw[b] = sw_pool.tile([1, S], f32, tag="sw", name=f"sw{b}")
            nc.vector.tensor_copy(
                out=sw[b][:, T * TCHUNK : (T + 1) * TCHUNK], in_=sc_ps
            )
            del th_tiles[c]
            if T == NC - 1:
                # all scores of batch b in SBUF: bounce through DRAM to
                # transpose to [P, NT] (token-within-tile on partitions)
                scr[b] = scr_pool.tile([1, S], f32, tag="scr", name=f"scr{b}")
                nc.sync.dma_start(out=scr[b], in_=sw[b])

        def emit_softmax(b):
            # transposed reload; softmax across all 128 lanes
            swT = sw_pool.tile([P, NT], f32, tag="swT")
            nc.sync.dma_start(
                out=swT, in_=scr[b].rearrange("o (n p) -> (o p) n", p=P)
            )
            asum = sw_pool.tile([P, 1], f32, tag="asum")
            eT = sw_pool.tile([P, NT], f32, tag="eT")
            nc.scalar.activation(
                out=eT, in_=swT, func=AF.Exp, scale=INV, accum_out=asum
            )
            tot_ps = ps_misc.tile([1, 1], f32, tag="misc")
            nc.tensor.matmul(tot_ps, lhsT=asum, rhs=ones_col, start=True, stop=True)
            rsum = sw_pool.tile([1, 1], f32, tag="rsum")
            nc.vector.reciprocal(out=rsum, in_=tot_ps)
            bc_ps = ps_misc.tile([P, 1], f32, tag="misc")
            nc.tensor.matmul(bc_ps, lhsT=ones_row, rhs=rsum, start=True, stop=True)
            wT[b] = sw_pool.tile([P, NT], f32, tag="wT", name=f"wT{b}")
            nc.vector.tensor_scalar_mul(out=wT[b], in0=eT, scalar1=bc_ps)

        def emit_scale_out(b, last):
            # out = weights * contexts; on the drain tail give ACT a 1/4
            # share (it is ~2.5x slower per tile than DVE at this op)
            for t in range(NT):
                ot = out_pool.tile([P, D], bf16, tag="ot")
                if last and t % 4 == 3:
                    nc.scalar.activation(
                        out=ot,
                        in_=xbf_sb[b][:, t, :],
                        func=AF.Copy,
                        scale=wT[b][:, t : t + 1],
                    )
                else:
                    nc.vector.tensor_scalar_mul(
                        out=ot, in0=xbf_sb[b][:, t, :], scalar1=wT[b][:, t : t + 1]
                    )
                nc.gpsimd.dma_start(out=out3[b * NT + t], in_=ot)

        dma_in_batch(0)
        if b_loc > 1:
            dma_in_batch(1)

        for c in range(NCH):
            b, T = divmod(c, NC)
            for ho in range(HO):
                emit_rc_group(b, T, ho)
            if c > 0:
                emit_matvec_block(c - 1)
            if T == 1 and b + 2 < b_loc:
                dma_in_batch(b + 2)
            if T == 1 and b > 0:
                emit_softmax(b - 1)
            if T == 2 and b > 0:
                emit_scale_out(b - 1, last=False)

        # drain: matvec for the last chunk, then batch b_loc-1 tail
        emit_matvec_block(NCH - 1)
        emit_softmax(b_loc - 1)
        emit_scale_out(b_loc - 1, last=True)


def build_module(b_loc=B_LOC):
    """Build and compile the Bacc module for one core (SPMD-replicated)."""
    import concourse.mybir as mybir
    import concourse.tile as tile
    from concourse import bacc

    f32 = mybir.dt.float32
    bf16 = mybir.dt.bfloat16
    f8e4 = mybir.dt.float8e4
    nc = bacc.Bacc("TRN2", target_bir_lowering=False, debug=False)

    ins = {
        "xt8": nc.dram_tensor(
            "xt8", [b_loc * NC * P, KO2 * 2 * TCHUNK], f8e4, kind="ExternalInput"
        ).ap(),
        "xbf": nc.dram_tensor("xbf", [b_loc * S, D], bf16, kind="ExternalInput").ap(),
        "w2dr": nc.dram_tensor("w2dr", [P, KO2, 2, H], f8e4, kind="ExternalInput").ap(),
        "w1dr": nc.dram_tensor("w1dr", [P, KO2, 2, H], f8e4, kind="ExternalInput").ap(),
        "ent8": nc.dram_tensor(
            "ent8", [P, KO2, 2, 16], f8e4, kind="ExternalInput"
        ).ap(),
        "v8": nc.dram_tensor("v8", [P, HO2, 2, 16], f8e4, kind="ExternalInput").ap(),
        "biasb": nc.dram_tensor("biasb", [P, HO], f32, kind="ExternalInput").ap(),
    }
    out_ap = nc.dram_tensor("out", [b_loc * S, D], bf16, kind="ExternalOutput").ap()

    with tile.TileContext(nc) as tc:
        build_attention(tc, out_ap, ins, b_loc=b_loc)

    nc.compile()
    return nc


_NC_CACHE = {}


def _get_module():
    key = B_LOC
    if key not in _NC_CACHE:
        _NC_CACHE[key] = build_module(key)
    return _NC_CACHE[key]


def _dr_interleave(w):
    """[D, N] -> [P, KO2, 2, N] with d = ko2*256 + two*128 + Ki."""
    n = w.shape[1]
    return np.ascontiguousarray(w.reshape(KO2, 2, P, n).transpose(2, 0, 1, 3))


def _prep_shared(inputs):
    w1 = np.asarray(inputs["w1_w"], np.float32)
    w2 = np.asarray(inputs["w2_w"], np.float32)
    b1 = np.asarray(inputs["w1_b"], np.float32)
    b2 = np.asarray(inputs["w2_b"], np.float32)
    v = np.asarray(inputs["v_w"], np.float32)
    w2dr = _dr_interleave(w2 * FP8_SCALE).astype(F8)
    w1dr = _dr_interleave(w1 * FP8_SCALE).astype(F8)
    biasb = np.ascontiguousarray((b1 + b2).reshape(HO, P).T)
    v8 = np.zeros([P, HO2, 2, 16], np.float32)
    v8[:, :, :, 0] = (v[:, 0] * FP8_SCALE).reshape(HO2, 2, P).transpose(2, 0, 1)
    return dict(w2dr=w2dr, w1dr=w1dr, biasb=biasb, v8=v8.astype(F8))


def make_in_maps(inputs):
    entities = np.asarray(inputs["entities"], np.float32)
    contexts = np.asarray(inputs["contexts"], np.float32)
    shared = _prep_shared(inputs)
    in_maps = []
    for c in range(N_CORES):
        ctx = contexts[c * B_LOC : (c + 1) * B_LOC].reshape(TOK, D)
        # [d, tok] -> [b, T, Ki, ko2, two, t] chunk-blocked fp8
        xt8 = (
            ctx.T.reshape(KO2, 2, P, B_LOC, NC, TCHUNK)
            .transpose(3, 4, 2, 0, 1, 5)
            .reshape(B_LOC * NC * P, KO2 * 2 * TCHUNK)
        )
        ent = entities[c * B_LOC : (c + 1) * B_LOC]
        ent8 = np.zeros([P, KO2, 2, 16], np.float32)
        ent8[:, :, :, :B_LOC] = _dr_interleave(np.ascontiguousarray(ent.T))
        in_maps.append(
            dict(
                xt8=np.ascontiguousarray(xt8).astype(F8),
                xbf=ctx.astype(BF16),
                ent8=ent8.astype(F8),
                **shared,
            )
        )
    return in_maps


def run(inputs, trace=False, **kwargs):
    """Run on all 8 cores; returns (full_output, BassKernelResults)."""
    from concourse.bass_utils import run_bass_kernel_spmd

    nc = _get_module()
    res = run_bass_kernel_spmd(
        nc, make_in_maps(inputs), core_ids=list(range(N_CORES)), trace=trace, **kwargs
    )
    out = np.concatenate(
        [
            res.results[c]["out"].astype(np.float32).reshape(B_LOC, S, D)
            for c in range(N_CORES)
        ],
        axis=0,
    )
    return out, res


def kernel(**inputs) -> np.ndarray:
    out, _ = run(inputs, trace=False)
    return out
